# revision 9
# baseline (speedup 1.0000x reference)
"""BrainRNN Trainium2 kernel: 8-core tensor-parallel Bass/Tile implementation.

Strategy (per sharding hint): shard every layer's 1024 output nodes across 8
cores (128 rows/core), all-gather the 32x128 activation shard each layer.

Key implementation choices:
  - Host-side sharding packs every weight block directly into PE-ready lhsT
    tile layout (contraction dim on partitions), f16, one contiguous
    [128, 7168] slab per (core, layer).  No on-chip transposes at all.
  - adj is packed int8 in the same slab layout; SWDGE cast-DMA (int8->f16)
    loads it, DVE applies the mask in place on the weight slab.
  - Structural zeros (shape-derived) are never loaded: Wr_m(k) columns
    [:(k+1)*1024] and W_s[j] padding columns [(j+1)*1024:] are dropped.
  - Node ownership is permuted: core c's slab row j is layer-local node
    (j%8)*128 + 16*c + j//8.  With that ordering the AllGather output
    [1024, 32] reads back into the canonical [128, 8*32] xxT tile layout
    with ONE fully contiguous DMA (512B per partition line).
  - Per layer the W_h matmuls run last so the previous layer's gather
    latency hides under W_s / W_r streaming; weight DMA runs ~2 layers
    ahead via triple-buffered slab pools.
  - Per-layer traffic/core: 1.83 MB f16 weights + 0.92 MB int8 adj.
"""

import sys

sys.path.insert(0, "/opt/trn_rl_repo")

import numpy as np

D = 1024
L = 8
N = 8192
B = 32
P = 128
NC = 8
CW = 7 * D  # weight-slab columns per layer

_CACHE = {}


def _perm(c):
    j = np.arange(P)
    return (j % 8) * 128 + 16 * c + j // 8


def _build(spmd=True, reps=1, ag=True, shared=True, adj_mode="i8", do_mask=True,
           wp_bufs=4, ap_bufs=4):
    import concourse.bacc as bacc
    import concourse.tile as tile
    import concourse.mybir as mybir

    F32 = mybir.dt.float32
    F16 = mybir.dt.float16
    I8 = mybir.dt.int8

    nc = bacc.Bacc(
        "TRN2", target_bir_lowering=False, debug=False, num_devices=NC if spmd else 1
    )

    # ---- DRAM I/O ------------------------------------------------------
    xt_d = nc.dram_tensor("xt", [P, 2 * B], F16, kind="ExternalInput")
    ht_d = nc.dram_tensor("ht", [P, 64 * B], F16, kind="ExternalInput")
    winT_d = nc.dram_tensor("winT", [P, 256], F16, kind="ExternalInput")
    bias_d = nc.dram_tensor("bias", [P, L], F32, kind="ExternalInput")
    woT_d = nc.dram_tensor("woT", [P, 8 * 64], F16, kind="ExternalInput")
    bo_d = nc.dram_tensor("bo", [64, 1], F32, kind="ExternalInput")
    wl_d = [
        nc.dram_tensor(f"wl{l}", [P, CW], F16, kind="ExternalInput") for l in range(L)
    ]
    ADT = I8 if adj_mode in ("i8", "i8raw") else F16
    al_d = (
        [nc.dram_tensor(f"al{l}", [P, CW], ADT, kind="ExternalInput") for l in range(L)]
        if adj_mode != "none"
        else [None] * L
    )
    outT_d = nc.dram_tensor("outT", [64, B], F32, kind="ExternalOutput")

    SIG = mybir.ActivationFunctionType.Sigmoid

    with tile.TileContext(nc) as tc:
        with (
            tc.tile_pool(name="cst", bufs=1) as cst,
            tc.tile_pool(name="xxp", bufs=2) as xxp,
            tc.tile_pool(name="wp", bufs=wp_bufs) as wp,
            tc.tile_pool(name="adp", bufs=ap_bufs) as adp,
            tc.tile_pool(name="psl", bufs=2, space="PSUM") as psl,
            tc.tile_pool(name="pso", bufs=1, space="PSUM") as pso,
            tc.tile_pool(name="dram", bufs=1, space="DRAM") as dram,
        ):
            # ---- constants -----------------------------------------------
            xt = cst.tile([P, 2 * B], F16, tag="xt")
            nc.sync.dma_start(xt[:], xt_d[:, :])
            ht = cst.tile([P, 64 * B], F16, tag="ht")
            nc.sync.dma_start(ht[:], ht_d[:, :])
            winT = cst.tile([P, 256], F16, tag="winT")
            nc.sync.dma_start(winT[:], winT_d[:, :])
            bias = cst.tile([P, L], F32, tag="bias")
            nc.sync.dma_start(bias[:], bias_d[:, :])
            woT = cst.tile([P, 8 * 64], F16, tag="woT")
            nc.sync.dma_start(woT[:], woT_d[:, :])
            bo = cst.tile([64, 1], F32, tag="bo")
            nc.sync.dma_start(bo[:], bo_d[:, :])

            # Software-pipelined emission: the sync/SP HWDGE ring and the
            # gpsimd SWDGE ring both execute DMAs FIFO per ring, so a
            # chain DMA (cci write / AllGather / xxT reload, all gated on
            # the previous layer's gather via sigmoid) would head-of-line
            # block later layers' weight/adj streams if emitted in naive
            # layer order.  Emit loads LOOKAHEAD layers ahead of each
            # layer's compute+chain group, across rep boundaries.
            LOOKAHEAD = 2
            xxTs = [[None] * L for _ in range(reps)]

            def stage(r, l):
                w_sl = wp.tile([P, CW], F16, tag="w", name=f"w{r}_{l}")
                HW = CW // 2
                for hh in range(2):
                    nc.sync.dma_start(
                        w_sl[:, hh * HW : (hh + 1) * HW],
                        wl_d[l][:, hh * HW : (hh + 1) * HW],
                    )
                if al_d[l] is not None:
                    # "i8": SWDGE cast-DMA int8->f16.  "i8raw": plain HWDGE
                    # int8 load, DVE does the mixed-dtype multiply.
                    a_sl = adp.tile(
                        [P, CW], I8 if adj_mode == "i8raw" else F16,
                        tag="a", name=f"a{r}_{l}",
                    )
                    eng = nc.gpsimd if adj_mode == "i8" else nc.sync
                    for hh in range(2):
                        eng.dma_start(
                            a_sl[:, hh * HW : (hh + 1) * HW],
                            al_d[l][:, hh * HW : (hh + 1) * HW],
                        )
                else:
                    a_sl = None
                return w_sl, a_sl

            def group(r, l, w_sl, a_sl):
                xxT = xxTs[r]
                acc = psl.tile([P, B], F32, tag="acc", name=f"acc{r}_{l}")
                nmm = 58 if l == 0 else 56
                nn = [0]

                def mm(lhsT, rhs):
                    nc.tensor.matmul(
                        acc[:, :], lhsT, rhs,
                        start=(nn[0] == 0), stop=(nn[0] == nmm - 1),
                    )
                    nn[0] += 1

                if l == 0:
                    mm(winT[:, 0:P], xt[:, 0:B])
                    mm(winT[:, P : 2 * P], xt[:, B : 2 * B])

                def rhs_of(tt):
                    # slab block order: W_s[l-2] sources 0..l-2, W_r[l]
                    # (sources (l+1)*D..N), W_h[l-1] last.
                    if l == 0:
                        return ht[:, (8 + tt) * B : (9 + tt) * B]
                    b, ti = tt // 8, tt % 8
                    nskip = l - 1
                    if b < nskip:
                        return xxT[b][:, ti * B : (ti + 1) * B]
                    if b < 6:
                        base = (l + 1) * 8 + (b - nskip) * 8
                        return ht[:, (base + ti) * B : (base + ti + 1) * B]
                    return xxT[l - 1][:, ti * B : (ti + 1) * B]

                for q in range(14):
                    sl = slice(q * 512, (q + 1) * 512)
                    if a_sl is not None and do_mask:
                        nc.vector.tensor_mul(w_sl[:, sl], w_sl[:, sl], a_sl[:, sl])
                    for t4 in range(4):
                        tt = q * 4 + t4
                        mm(w_sl[:, tt * P : (tt + 1) * P], rhs_of(tt))

                # ---- sigmoid(+bias), allgather, reload --------------------
                xs = cst.tile([P, B], F16, tag="xs", name=f"xs{r}_{l}")
                nc.scalar.activation(
                    xs[:], acc[:, :], SIG, bias=bias[:, l : l + 1], scale=1.0
                )
                cci = dram.tile([P, B], F16, tag=f"cci{l}", name=f"cci{r}_{l}")
                cco = dram.tile(
                    [NC * P, B], F16, tag=f"cco{l}", name=f"cco{r}_{l}",
                    addr_space="Shared" if (shared and spmd and ag) else "Local",
                )
                nc.sync.dma_start(cci[:], xs[:])
                if spmd and ag:
                    nc.gpsimd.collective_compute(
                        "AllGather",
                        mybir.AluOpType.bypass,
                        replica_groups=[list(range(NC))],
                        ins=[cci[:].opt()],
                        outs=[cco[:].opt()],
                    )
                else:
                    nc.sync.dma_start(cco[0:P, :], cci[:])
                xxT[l] = xxp.tile([P, 8 * B], F16, tag=f"xxT{l}", name=f"xxT{r}_{l}")
                nc.sync.dma_start(
                    xxT[l][:], cco[:].rearrange("(p s) b -> p (s b)", p=P)
                )

            def out_group(r):
                xxT = xxTs[r]
                ops = pso.tile([P, B], F32, tag="ops", name=f"ops{r}")
                for t in range(8):
                    nc.tensor.matmul(
                        ops[:64, :],
                        woT[:, t * 64 : (t + 1) * 64],
                        xxT[7][:, t * B : (t + 1) * B],
                        start=(t == 0),
                        stop=(t == 7),
                    )
                outT = cst.tile([64, B], F32, tag="outT", name=f"outT{r}")
                nc.vector.tensor_scalar_add(outT[:], ops[:64, :], bo[:, 0:1])
                nc.sync.dma_start(outT_d[:, :], outT[:])

            items = [(r, l) for r in range(reps) for l in range(L)]
            loaded = {}
            pend_out = []
            for i, (r, l) in enumerate(items):
                loaded[(r, l)] = stage(r, l)
                if i >= LOOKAHEAD:
                    rg, lg = items[i - LOOKAHEAD]
                    group(rg, lg, *loaded.pop((rg, lg)))
                    if lg == L - 1:
                        pend_out.append(rg)
                    elif pend_out:
                        out_group(pend_out.pop(0))
            for j, (rg, lg) in enumerate(items[-LOOKAHEAD:] if LOOKAHEAD else []):
                group(rg, lg, *loaded.pop((rg, lg)))
                if lg == L - 1:
                    pend_out.append(rg)
            for rg in pend_out:
                out_group(rg)

    nc.compile()
    return nc


def _pack_nk(m, k):
    """[C, k] (contraction-major) -> [128, (C//128)*k] slab:
    slab[p, t*k + j] = m[t*128 + p, j]."""
    C = m.shape[0]
    return np.ascontiguousarray(
        m.reshape(C // P, P, k).transpose(1, 0, 2).reshape(P, -1)
    )


def _pack_jn(m):
    """[128, C] (row-major weights) -> [128, C] lhsT slab:
    slab[p, t*128 + j] = m[j, t*128 + p]."""
    C = m.shape[1]
    return np.ascontiguousarray(
        m.reshape(P, C // P, P).transpose(2, 1, 0).reshape(P, C)
    )


def _layer_blocks(l):
    """Column-block source ranges (global neuron ids) in slab order."""
    if l == 0:
        return [(k * D, "r", 0) for k in range(1, 8)]
    blocks = [(mb * D, "s", l - 2) for mb in range(l - 1)]
    if l <= 6:
        blocks += [(k * D, "r", l) for k in range(l + 1, 8)]
    blocks += [((l - 1) * D, "h", l - 1)]
    return blocks


def _shard_inputs(inputs):
    x = np.asarray(inputs["x"], dtype=np.float32)
    h = np.asarray(inputs["hidden_states"], dtype=np.float32)
    adj = np.asarray(inputs["adj"])
    W_in = np.asarray(inputs["W_in"], dtype=np.float32)
    b_in = np.asarray(inputs["b_in"], dtype=np.float32)
    W_h = np.asarray(inputs["W_h"], dtype=np.float32)
    b_h = np.asarray(inputs["b_h"], dtype=np.float32)
    W_r = np.asarray(inputs["W_r"], dtype=np.float32)
    W_s = np.asarray(inputs["W_s"], dtype=np.float32)
    W_o = np.asarray(inputs["W_o"], dtype=np.float32)
    b_o = np.asarray(inputs["b_o"], dtype=np.float32)

    adjT8 = np.ascontiguousarray(adj.T).astype(np.int8)  # [target, source]

    ht = _pack_nk(h.T.astype(np.float16), B)
    xt = _pack_nk(x.T.astype(np.float16), B)
    woT = _pack_nk(W_o.T.astype(np.float16), 64)
    bo = np.ascontiguousarray(b_o).reshape(64, 1)

    maps = []
    for c in range(NC):
        p = _perm(c)
        winT = _pack_jn(W_in[p].astype(np.float16))
        bias = np.stack([b_in[p]] + [b_h[i][p] for i in range(7)], axis=1)
        bias = np.ascontiguousarray(bias, dtype=np.float32)
        m = {
            "xt": xt, "ht": ht, "winT": winT, "bias": bias, "woT": woT, "bo": bo,
        }
        for l in range(L):
            wrows = []
            arows = []
            at = adjT8[l * D + p]  # [128, 8192] int8, mask rows for this shard
            for src0, kind, idx in _layer_blocks(l):
                if kind == "r":
                    wrows.append(W_r[idx][p][:, src0 : src0 + D])
                elif kind == "s":
                    wrows.append(W_s[idx][p][:, src0 : src0 + D])
                else:
                    wrows.append(W_h[idx][p])
                arows.append(at[:, src0 : src0 + D])
            wl = _pack_jn(np.concatenate(wrows, axis=1).astype(np.float16))
            al = _pack_jn(np.concatenate(arows, axis=1))
            m[f"wl{l}"] = wl
            m[f"al{l}"] = al
        maps.append(m)
    return maps


def get_compiled():
    if "nc" not in _CACHE:
        _CACHE["nc"] = _build()
    return _CACHE["nc"]


def run(inputs, **run_kwargs):
    from concourse import bass_utils

    nc = get_compiled()
    in_maps = _shard_inputs(inputs)
    res = bass_utils.run_bass_kernel_spmd(
        nc, in_maps, core_ids=list(range(NC)), **run_kwargs
    )
    out = np.ascontiguousarray(res.results[0]["outT"].T.astype(np.float32))
    return out, res


def kernel(**inputs):
    out, _ = run(inputs)
    return out


# revision 21
# speedup vs baseline: 1.0049x; 1.0049x over previous
"""BrainRNN Trainium2 kernel: 8-core tensor-parallel Bass/Tile implementation.

Strategy (per sharding hint): shard every layer's 1024 output nodes across 8
cores (128 rows/core), all-gather the 32x128 activation shard each layer.

Key implementation choices:
  - Host-side sharding packs every weight block directly into PE-ready lhsT
    tile layout (contraction dim on partitions), f16, one contiguous
    [128, 7168] slab per (core, layer).  No on-chip transposes at all.
  - adj is packed int8 in the same slab layout; SWDGE cast-DMA (int8->f16)
    loads it, DVE applies the mask in place on the weight slab.
  - Structural zeros (shape-derived) are never loaded: Wr_m(k) columns
    [:(k+1)*1024] and W_s[j] padding columns [(j+1)*1024:] are dropped.
  - Node ownership is permuted: core c's slab row j is layer-local node
    (j%8)*128 + 16*c + j//8.  With that ordering the AllGather output
    [1024, 32] reads back into the canonical [128, 8*32] xxT tile layout
    with ONE fully contiguous DMA (512B per partition line).
  - Per layer the W_h matmuls run last so the previous layer's gather
    latency hides under W_s / W_r streaming; weight DMA runs ~2 layers
    ahead via triple-buffered slab pools.
  - Per-layer traffic/core: 1.83 MB f16 weights + 0.92 MB int8 adj.
"""

import sys

sys.path.insert(0, "/opt/trn_rl_repo")

import numpy as np

D = 1024
L = 8
N = 8192
B = 32
P = 128
NC = 8
CW = 7 * D  # weight-slab columns per layer

_CACHE = {}


def _perm(c):
    j = np.arange(P)
    return (j % 8) * 128 + 16 * c + j // 8


def _build(spmd=True, reps=1, ag=True, shared=True, adj_mode="i8", do_mask=True,
           wp_bufs=4, ap_bufs=4, chain=True):
    import concourse.bacc as bacc
    import concourse.tile as tile
    import concourse.mybir as mybir

    F32 = mybir.dt.float32
    F16 = mybir.dt.float16
    I8 = mybir.dt.int8

    nc = bacc.Bacc(
        "TRN2", target_bir_lowering=False, debug=False, num_devices=NC if spmd else 1
    )

    # ---- DRAM I/O ------------------------------------------------------
    xt_d = nc.dram_tensor("xt", [P, 2 * B], F16, kind="ExternalInput")
    ht_d = nc.dram_tensor("ht", [P, 64 * B], F16, kind="ExternalInput")
    winT_d = nc.dram_tensor("winT", [P, 256], F16, kind="ExternalInput")
    bias_d = nc.dram_tensor("bias", [P, L], F32, kind="ExternalInput")
    woT_d = nc.dram_tensor("woT", [P, 8 * 64], F16, kind="ExternalInput")
    bo_d = nc.dram_tensor("bo", [64, 1], F32, kind="ExternalInput")
    wl_d = [
        nc.dram_tensor(f"wl{l}", [P, CW], F16, kind="ExternalInput") for l in range(L)
    ]
    ADT = I8 if adj_mode in ("i8", "i8raw") else F16
    al_d = (
        [nc.dram_tensor(f"al{l}", [P, CW], ADT, kind="ExternalInput") for l in range(L)]
        if adj_mode != "none"
        else [None] * L
    )
    outT_d = nc.dram_tensor("outT", [64, B], F32, kind="ExternalOutput")

    SIG = mybir.ActivationFunctionType.Sigmoid

    with tile.TileContext(nc) as tc:
        with (
            tc.tile_pool(name="cst", bufs=1) as cst,
            tc.tile_pool(name="xxp", bufs=2) as xxp,
            tc.tile_pool(name="wp", bufs=wp_bufs) as wp,
            tc.tile_pool(name="adp", bufs=ap_bufs) as adp,
            tc.tile_pool(name="whp", bufs=6) as whp,
            tc.tile_pool(name="ahp", bufs=6) as ahp,
            tc.tile_pool(name="psl", bufs=2, space="PSUM") as psl,
            tc.tile_pool(name="pso", bufs=1, space="PSUM") as pso,
            tc.tile_pool(name="dram", bufs=1, space="DRAM") as dram,
        ):
            # ---- constants -----------------------------------------------
            xt = cst.tile([P, 2 * B], F16, tag="xt")
            nc.sync.dma_start(xt[:], xt_d[:, :])
            ht = cst.tile([P, 64 * B], F16, tag="ht")
            nc.sync.dma_start(ht[:], ht_d[:, :])
            winT = cst.tile([P, 256], F16, tag="winT")
            nc.sync.dma_start(winT[:], winT_d[:, :])
            bias = cst.tile([P, L], F32, tag="bias")
            nc.sync.dma_start(bias[:], bias_d[:, :])
            woT = cst.tile([P, 8 * 64], F16, tag="woT")
            nc.sync.dma_start(woT[:], woT_d[:, :])
            bo = cst.tile([64, 1], F32, tag="bo")
            nc.sync.dma_start(bo[:], bo_d[:, :])

            # Software-pipelined emission: the sync/SP HWDGE ring and the
            # gpsimd SWDGE ring both execute DMAs FIFO per ring, so a
            # chain DMA (cci write / AllGather / xxT reload, all gated on
            # the previous layer's gather via sigmoid) would head-of-line
            # block later layers' weight/adj streams if emitted in naive
            # layer order.  Emit loads LOOKAHEAD layers ahead of each
            # layer's compute+chain group, across rep boundaries.
            LOOKAHEAD = 2
            xxTs = [[None] * L for _ in range(reps)]
            dummy = None
            if not chain:
                dummy = cst.tile([P, 8 * B], F16, tag="dummy")
                nc.sync.dma_start(dummy[:], ht_d[:, 0 : 8 * B])

            # The last 1024-col block of every layer slab is W_h -- its
            # matmuls are the only consumers gated on the FRESH gather of
            # the previous layer.  Keep it in its own small, deep pool so
            # the main slab's buffers recycle at stream rate instead of at
            # gather-chain rate.
            CM = 6 * D  # main-slab columns

            def stage(r, l):
                w_sl = wp.tile([P, CM], F16, tag="w", name=f"w{r}_{l}")
                HW = CM // 2
                for hh in range(2):
                    nc.sync.dma_start(
                        w_sl[:, hh * HW : (hh + 1) * HW],
                        wl_d[l][:, hh * HW : (hh + 1) * HW],
                    )
                wh_sl = whp.tile([P, D], F16, tag="wh", name=f"wh{r}_{l}")
                nc.sync.dma_start(wh_sl[:], wl_d[l][:, CM:CW])
                if al_d[l] is not None:
                    # "i8": SWDGE cast-DMA int8->f16.  "i8raw": plain HWDGE
                    # int8 load, DVE does the mixed-dtype multiply.
                    ADTS = I8 if adj_mode == "i8raw" else F16
                    eng = nc.gpsimd if adj_mode == "i8" else nc.sync
                    a_sl = adp.tile([P, CM], ADTS, tag="a", name=f"a{r}_{l}")
                    for hh in range(2):
                        eng.dma_start(
                            a_sl[:, hh * HW : (hh + 1) * HW],
                            al_d[l][:, hh * HW : (hh + 1) * HW],
                        )
                    ah_sl = ahp.tile([P, D], ADTS, tag="ah", name=f"ah{r}_{l}")
                    eng.dma_start(ah_sl[:], al_d[l][:, CM:CW])
                else:
                    a_sl, ah_sl = None, None
                return w_sl, a_sl, wh_sl, ah_sl

            def group(r, l, w_sl, a_sl, wh_sl, ah_sl):
                xxT = xxTs[r]
                acc = psl.tile([P, B], F32, tag="acc", name=f"acc{r}_{l}")
                nmm = 58 if l == 0 else 56
                nn = [0]

                def mm(lhsT, rhs):
                    nc.tensor.matmul(
                        acc[:, :], lhsT, rhs,
                        start=(nn[0] == 0), stop=(nn[0] == nmm - 1),
                    )
                    nn[0] += 1

                if l == 0:
                    mm(winT[:, 0:P], xt[:, 0:B])
                    mm(winT[:, P : 2 * P], xt[:, B : 2 * B])

                def rhs_of(tt):
                    # slab block order: W_s[l-2] sources 0..l-2, W_r[l]
                    # (sources (l+1)*D..N), W_h[l-1] last.
                    if l == 0:
                        return ht[:, (8 + tt) * B : (9 + tt) * B]
                    b, ti = tt // 8, tt % 8
                    nskip = l - 1
                    if b < nskip:
                        return xxT[b][:, ti * B : (ti + 1) * B]
                    if b < 6:
                        base = (l + 1) * 8 + (b - nskip) * 8
                        return ht[:, (base + ti) * B : (base + ti + 1) * B]
                    return xxT[l - 1][:, ti * B : (ti + 1) * B]

                for q in range(14):
                    if q < 12:
                        wt, at, base = w_sl, a_sl, 0
                    else:
                        wt, at, base = wh_sl, ah_sl, CM
                    sl = slice(q * 512 - base, (q + 1) * 512 - base)
                    if at is not None and do_mask:
                        nc.vector.tensor_mul(wt[:, sl], wt[:, sl], at[:, sl])
                    for t4 in range(4):
                        tt = q * 4 + t4
                        off = tt * P - base
                        mm(wt[:, off : off + P], rhs_of(tt))

                # ---- sigmoid(+bias), allgather, reload --------------------
                if not chain:
                    # timing-ablation: no activation chain at all; consumers
                    # read a resident dummy tile (results are wrong).
                    xxT[l] = dummy
                    return
                xs = cst.tile([P, B], F16, tag="xs", name=f"xs{r}_{l}")
                nc.scalar.activation(
                    xs[:], acc[:, :], SIG, bias=bias[:, l : l + 1], scale=1.0
                )
                cci = dram.tile([P, B], F16, tag=f"cci{l}", name=f"cci{r}_{l}")
                cco = dram.tile(
                    [NC * P, B], F16, tag=f"cco{l}", name=f"cco{r}_{l}",
                    addr_space="Shared" if (shared and spmd and ag) else "Local",
                )
                nc.sync.dma_start(cci[:], xs[:])
                if spmd and ag:
                    nc.gpsimd.collective_compute(
                        "AllGather",
                        mybir.AluOpType.bypass,
                        replica_groups=[list(range(NC))],
                        ins=[cci[:].opt()],
                        outs=[cco[:].opt()],
                    )
                else:
                    nc.sync.dma_start(cco[0:P, :], cci[:])
                xxT[l] = xxp.tile([P, 8 * B], F16, tag=f"xxT{l}", name=f"xxT{r}_{l}")
                nc.sync.dma_start(
                    xxT[l][:], cco[:].rearrange("(p s) b -> p (s b)", p=P)
                )

            def out_group(r):
                xxT = xxTs[r]
                ops = pso.tile([P, B], F32, tag="ops", name=f"ops{r}")
                for t in range(8):
                    nc.tensor.matmul(
                        ops[:64, :],
                        woT[:, t * 64 : (t + 1) * 64],
                        xxT[7][:, t * B : (t + 1) * B],
                        start=(t == 0),
                        stop=(t == 7),
                    )
                outT = cst.tile([64, B], F32, tag="outT", name=f"outT{r}")
                nc.vector.tensor_scalar_add(outT[:], ops[:64, :], bo[:, 0:1])
                nc.sync.dma_start(outT_d[:, :], outT[:])

            items = [(r, l) for r in range(reps) for l in range(L)]
            loaded = {}
            pend_out = []
            for i, (r, l) in enumerate(items):
                loaded[(r, l)] = stage(r, l)
                if i >= LOOKAHEAD:
                    rg, lg = items[i - LOOKAHEAD]
                    group(rg, lg, *loaded.pop((rg, lg)))
                    if lg == L - 1:
                        pend_out.append(rg)
                    elif pend_out:
                        out_group(pend_out.pop(0))
            for j, (rg, lg) in enumerate(items[-LOOKAHEAD:] if LOOKAHEAD else []):
                group(rg, lg, *loaded.pop((rg, lg)))
                if lg == L - 1:
                    pend_out.append(rg)
            for rg in pend_out:
                out_group(rg)

    nc.compile()
    return nc


def _pack_nk(m, k):
    """[C, k] (contraction-major) -> [128, (C//128)*k] slab:
    slab[p, t*k + j] = m[t*128 + p, j]."""
    C = m.shape[0]
    return np.ascontiguousarray(
        m.reshape(C // P, P, k).transpose(1, 0, 2).reshape(P, -1)
    )


def _pack_jn(m):
    """[128, C] (row-major weights) -> [128, C] lhsT slab:
    slab[p, t*128 + j] = m[j, t*128 + p]."""
    C = m.shape[1]
    return np.ascontiguousarray(
        m.reshape(P, C // P, P).transpose(2, 1, 0).reshape(P, C)
    )


def _layer_blocks(l):
    """Column-block source ranges (global neuron ids) in slab order."""
    if l == 0:
        return [(k * D, "r", 0) for k in range(1, 8)]
    blocks = [(mb * D, "s", l - 2) for mb in range(l - 1)]
    if l <= 6:
        blocks += [(k * D, "r", l) for k in range(l + 1, 8)]
    blocks += [((l - 1) * D, "h", l - 1)]
    return blocks


def _shard_inputs(inputs):
    x = np.asarray(inputs["x"], dtype=np.float32)
    h = np.asarray(inputs["hidden_states"], dtype=np.float32)
    adj = np.asarray(inputs["adj"])
    W_in = np.asarray(inputs["W_in"], dtype=np.float32)
    b_in = np.asarray(inputs["b_in"], dtype=np.float32)
    W_h = np.asarray(inputs["W_h"], dtype=np.float32)
    b_h = np.asarray(inputs["b_h"], dtype=np.float32)
    W_r = np.asarray(inputs["W_r"], dtype=np.float32)
    W_s = np.asarray(inputs["W_s"], dtype=np.float32)
    W_o = np.asarray(inputs["W_o"], dtype=np.float32)
    b_o = np.asarray(inputs["b_o"], dtype=np.float32)

    adjT8 = np.ascontiguousarray(adj.T).astype(np.int8)  # [target, source]

    ht = _pack_nk(h.T.astype(np.float16), B)
    xt = _pack_nk(x.T.astype(np.float16), B)
    woT = _pack_nk(W_o.T.astype(np.float16), 64)
    bo = np.ascontiguousarray(b_o).reshape(64, 1)

    maps = []
    for c in range(NC):
        p = _perm(c)
        winT = _pack_jn(W_in[p].astype(np.float16))
        bias = np.stack([b_in[p]] + [b_h[i][p] for i in range(7)], axis=1)
        bias = np.ascontiguousarray(bias, dtype=np.float32)
        m = {
            "xt": xt, "ht": ht, "winT": winT, "bias": bias, "woT": woT, "bo": bo,
        }
        for l in range(L):
            wrows = []
            arows = []
            at = adjT8[l * D + p]  # [128, 8192] int8, mask rows for this shard
            for src0, kind, idx in _layer_blocks(l):
                if kind == "r":
                    wrows.append(W_r[idx][p][:, src0 : src0 + D])
                elif kind == "s":
                    wrows.append(W_s[idx][p][:, src0 : src0 + D])
                else:
                    wrows.append(W_h[idx][p])
                arows.append(at[:, src0 : src0 + D])
            wl = _pack_jn(np.concatenate(wrows, axis=1).astype(np.float16))
            al = _pack_jn(np.concatenate(arows, axis=1))
            m[f"wl{l}"] = wl
            m[f"al{l}"] = al
        maps.append(m)
    return maps


def get_compiled():
    if "nc" not in _CACHE:
        _CACHE["nc"] = _build()
    return _CACHE["nc"]


def run(inputs, **run_kwargs):
    from concourse import bass_utils

    nc = get_compiled()
    in_maps = _shard_inputs(inputs)
    res = bass_utils.run_bass_kernel_spmd(
        nc, in_maps, core_ids=list(range(NC)), **run_kwargs
    )
    out = np.ascontiguousarray(res.results[0]["outT"].T.astype(np.float32))
    return out, res


def kernel(**inputs):
    out, _ = run(inputs)
    return out
